# revision 7
# baseline (speedup 1.0000x reference)
"""Causal self-attention (B=2, T=2048, C=1024, NH=16) on 8 TRN2 NeuronCores.

Sharding: pure head-parallel — core j owns heads {2j, 2j+1} for BOTH batches.
Each core computes qkv (transposed layouts) for its heads over all 4096 rows,
runs causal attention for its 4 (batch, head) instances, then the cores
exchange attention outputs with a single 8-way AllToAll so that core j ends
up with all 1024 channels for global rows [512j, 512j+512).  The projection
is then row-parallel (each core multiplies its row slice by the full w_proj)
and the host just concatenates/transposes the per-core output slices.

All matmuls run as float32r (full-rate fp32 mode on the PE, ~13 mantissa
bits); softmax runs without max-subtraction (scores are O(5), exp is safe in
fp32) and the normalization is folded in after the PV matmul, whose stationary
operand carries an extra ones-column so the softmax denominator comes out of
the same accumulation for free.
"""

import sys

sys.path.insert(0, "/opt/trn_rl_repo")

import numpy as np

import concourse.bass as bass
import concourse.mybir as mybir
from concourse import bacc, tile
from concourse import bass_utils
from concourse.masks import make_identity

B, T, C = 2, 2048, 1024
NH, HD = 16, 64
R = B * T                 # 4096 global rows
P = 128
NCORES = 8
SCALE = 0.125             # 1/sqrt(HD)
CC = C // P               # 8 contraction chunks
RC = 8                    # row chunks of 512
RCH = R // RC             # 512
QC = 4                    # q chunks of 512 per batch
KT = T // P               # 16 k-tiles of 128 per batch
NEG = -1.0e9

f32 = mybir.dt.float32
f32r = mybir.dt.float32r

_PROGRAM = None


def _build_program(repeat=1, collective=True, num_devices=NCORES):
    nc = bacc.Bacc("TRN2", target_bir_lowering=False, debug=False,
                   num_devices=num_devices)
    xT_ap = nc.dram_tensor("xT", [C, R], f32r, kind="ExternalInput").ap()
    wqkv_ap = nc.dram_tensor("wqkv", [C, 3 * P], f32r, kind="ExternalInput").ap()
    wproj_ap = nc.dram_tensor("wproj", [C, C], f32r, kind="ExternalInput").ap()
    outT_ap = nc.dram_tensor("outT", [C, RCH], f32, kind="ExternalOutput").ap()

    with tile.TileContext(nc) as tc:
        for _rep in range(repeat):
            _emit_body(tc, nc, xT_ap, wqkv_ap, wproj_ap, outT_ap, collective)

    nc.compile()
    return nc


def _emit_body(tc, nc, xT_ap, wqkv_ap, wproj_ap, outT_ap, collective=True):
        with tc.tile_pool(name="const", bufs=1) as const, \
             tc.tile_pool(name="wq", bufs=1) as wqp, \
             tc.tile_pool(name="wp", bufs=1) as wpp, \
             tc.tile_pool(name="xt", bufs=10) as xtp, \
             tc.tile_pool(name="qkv", bufs=1) as qkvp, \
             tc.tile_pool(name="vtmp", bufs=3) as vtmpp, \
             tc.tile_pool(name="vo", bufs=1) as vop, \
             tc.tile_pool(name="yt", bufs=1) as ytp, \
             tc.tile_pool(name="exp", bufs=16) as expp, \
             tc.tile_pool(name="small", bufs=4) as smallp, \
             tc.tile_pool(name="outsb", bufs=2) as outsbp, \
             tc.tile_pool(name="mm512", bufs=2, space="PSUM") as ps_mm, \
             tc.tile_pool(name="ps_tr", bufs=1, space="PSUM") as ps_tr, \
             tc.tile_pool(name="ps_s", bufs=4, space="PSUM") as ps_s, \
             tc.tile_pool(name="ps_y", bufs=1, space="PSUM") as ps_y, \
             tc.tile_pool(name="dram", bufs=1, space="DRAM") as dram:

            # ---- constants -------------------------------------------------
            ident = const.tile([P, P], f32)
            make_identity(nc, ident[:])
            ones = const.tile([P, 1], f32)
            nc.gpsimd.memset(ones[:], 1.0)
            masks = []
            for d in range(4):
                m = const.tile([P, RCH], f32, name=f"mask{d}")
                nc.gpsimd.memset(m[:], 0.0)
                # keep where q - k = f - p - 128*d >= 0, else NEG
                nc.gpsimd.affine_select(
                    out=m[:], in_=m[:], compare_op=mybir.AluOpType.is_ge,
                    fill=NEG, base=-128 * d, pattern=[[1, RCH]],
                    channel_multiplier=-1)
                masks.append(m)

            # ---- weights ---------------------------------------------------
            wq = wqp.tile([P, CC, 3 * P], f32r)
            nc.sync.dma_start(wq[:], wqkv_ap.rearrange("(co ci) n -> ci co n", ci=P))
            wp = wpp.tile([P, CC, C], f32r)
            nc.sync.dma_start(wp[:], wproj_ap.rearrange("(co ci) n -> ci co n", ci=P))

            # ---- phase 1: qkv ---------------------------------------------
            qT = qkvp.tile([P, R], f32r, name="qT")
            kT = qkvp.tile([P, R], f32r, name="kT")
            vo = vop.tile([P, 2 * KT, 130], f32r)  # [V_h0 | 1 | V_h1 | 1] per k-tile
            # ones columns of vo
            nc.vector.tensor_copy(vo[:, :, 64:65], ones[:, None, :].to_broadcast((P, 2 * KT, 1)))
            nc.vector.tensor_copy(vo[:, :, 129:130], ones[:, None, :].to_broadcast((P, 2 * KT, 1)))

            for rc in range(RC):
                xts = []
                for cc in range(CC):
                    xt = xtp.tile([P, RCH], f32r, tag="xt")
                    nc.sync.dma_start(
                        xt[:], xT_ap[P * cc:P * (cc + 1), RCH * rc:RCH * (rc + 1)])
                    xts.append(xt)
                for ct in range(3):
                    ps = ps_mm.tile([P, RCH], f32, tag="mm")
                    for cc in range(CC):
                        nc.tensor.matmul(ps[:], wq[:, cc, P * ct:P * (ct + 1)],
                                         xts[cc][:], start=(cc == 0),
                                         stop=(cc == CC - 1))
                    if ct == 0:
                        nc.vector.tensor_copy(qT[:, RCH * rc:RCH * (rc + 1)], ps[:])
                    elif ct == 1:
                        nc.vector.tensor_copy(kT[:, RCH * rc:RCH * (rc + 1)], ps[:])
                    else:
                        # v^T chunk -> transpose to natural V, pack into vo
                        vt = vtmpp.tile([P, RCH], f32)
                        nc.scalar.copy(vt[:], ps[:])
                        for s in range(RCH // P):
                            kt32 = 4 * rc + s  # global k-tile index (0..31)
                            pst = ps_tr.tile([P, P], f32)
                            nc.tensor.transpose(pst[:], vt[:, P * s:P * (s + 1)],
                                                ident[:])
                            nc.vector.tensor_copy(vo[:, kt32, 0:64], pst[:, 0:64])
                            nc.vector.tensor_copy(vo[:, kt32, 65:129], pst[:, 64:128])

            # ---- phase 2: attention ---------------------------------------
            yT = ytp.tile([P, R], f32r)
            for g in range(B):
                for h in range(2):
                    pr = 64 * h
                    for qc in range(QC):
                        qoff = T * g + RCH * qc
                        nkt = 4 * qc + 4
                        exps = []
                        for kt in range(nkt):
                            koff = T * g + P * kt
                            ps = ps_s.tile([P, RCH], f32)
                            nc.tensor.matmul(
                                ps[:], kT[pr:pr + 64, koff:koff + P],
                                qT[pr:pr + 64, qoff:qoff + RCH],
                                start=True, stop=True)
                            d = kt - 4 * qc
                            if d >= 0:
                                nc.vector.tensor_add(ps[:], ps[:], masks[d][:])
                            e = expp.tile([P, RCH], f32r, tag="exp")
                            nc.scalar.activation(
                                e[:], ps[:], mybir.ActivationFunctionType.Exp,
                                scale=SCALE)
                            exps.append(e)
                        psy = ps_y.tile([65, RCH], f32)
                        for kt in range(nkt):
                            nc.tensor.matmul(
                                psy[:], vo[:, KT * g + kt, 65 * h:65 * h + 65],
                                exps[kt][:], start=(kt == 0),
                                stop=(kt == nkt - 1))
                        rcp = smallp.tile([1, RCH], f32, tag="recip")
                        nc.vector.reciprocal(rcp[:], psy[64:65, :])
                        bc = smallp.tile([64, RCH], f32, tag="bcast")
                        nc.gpsimd.partition_broadcast(bc[:], rcp[:])
                        nc.vector.tensor_mul(yT[pr:pr + 64, qoff:qoff + RCH],
                                             psy[0:64, :], bc[:])

            # ---- all-to-all ------------------------------------------------
            a2a_in = dram.tile([C, RCH], f32r)
            a2a_out = dram.tile([C, RCH], f32r)
            for i in range(NCORES):
                nc.sync.dma_start(a2a_in[P * i:P * (i + 1), :],
                                  yT[:, RCH * i:RCH * (i + 1)])
            if collective:
                nc.gpsimd.collective_compute(
                    "AllToAll", mybir.AluOpType.bypass,
                    replica_groups=[list(range(NCORES))],
                    ins=[a2a_in.opt()], outs=[a2a_out.opt()])
            else:
                nc.sync.dma_start(a2a_out[:], a2a_in[:])

            # ---- phase 3: projection --------------------------------------
            ytms = []
            for cc in range(CC):
                ytm = xtp.tile([P, RCH], f32r, tag="xt")
                nc.sync.dma_start(ytm[:], a2a_out[P * cc:P * (cc + 1), :])
                ytms.append(ytm)
            for ct in range(CC):
                pp = ps_mm.tile([P, RCH], f32, tag="mm")
                for cc in range(CC):
                    nc.tensor.matmul(pp[:], wp[:, cc, P * ct:P * (ct + 1)],
                                     ytms[cc][:], start=(cc == 0),
                                     stop=(cc == CC - 1))
                ot = outsbp.tile([P, RCH], f32)
                if ct % 2 == 0:
                    nc.vector.tensor_copy(ot[:], pp[:])
                else:
                    nc.scalar.copy(ot[:], pp[:])
                nc.sync.dma_start(outT_ap[P * ct:P * (ct + 1), :], ot[:])


def _get_program():
    global _PROGRAM
    if _PROGRAM is None:
        _PROGRAM = _build_program()
    return _PROGRAM


def make_in_maps(x, w_qkv, w_proj):
    """Host-side sharding: build the 8 per-core input maps."""
    x = np.asarray(x, dtype=np.float32)
    w_qkv = np.asarray(w_qkv, dtype=np.float32)
    w_proj = np.asarray(w_proj, dtype=np.float32)
    xT = np.ascontiguousarray(x.reshape(R, C).T)            # (1024, 4096)
    w_proj = np.ascontiguousarray(w_proj)                   # (1024, 1024)
    in_maps = []
    for j in range(NCORES):
        h0 = 2 * j * HD                                     # first head col
        wq = w_qkv[:, h0:h0 + 2 * HD]
        wk = w_qkv[:, C + h0:C + h0 + 2 * HD]
        wv = w_qkv[:, 2 * C + h0:2 * C + h0 + 2 * HD]
        wshard = np.ascontiguousarray(np.concatenate([wq, wk, wv], axis=1))
        in_maps.append({"xT": xT, "wqkv": wshard, "wproj": w_proj})
    return in_maps


def assemble(results):
    """Host-side unshard: concatenate per-core transposed row slices."""
    y = np.empty((R, C), dtype=np.float32)
    for j in range(NCORES):
        y[RCH * j:RCH * (j + 1), :] = results[j]["outT"].T
    return y.reshape(B, T, C)


def kernel(x, w_qkv, w_proj):
    nc = _get_program()
    in_maps = make_in_maps(x, w_qkv, w_proj)
    res = bass_utils.run_bass_kernel_spmd(nc, in_maps,
                                          core_ids=list(range(NCORES)))
    return assemble(res.results)


# revision 8
# speedup vs baseline: 1.3120x; 1.3120x over previous
"""Causal self-attention (B=2, T=2048, C=1024, NH=16) on 8 TRN2 NeuronCores.

Sharding: pure head-parallel — core j owns heads {2j, 2j+1} for BOTH batches.
Each core computes qkv (transposed layouts) for its heads over all 4096 rows,
runs causal attention for its 4 (batch, head) instances, then the cores
exchange attention outputs with a single 8-way AllToAll so that core j ends
up with all 1024 channels for global rows [512j, 512j+512).  The projection
is then row-parallel (each core multiplies its row slice by the full w_proj)
and the host just concatenates/transposes the per-core output slices.

All matmuls run as float32r (full-rate fp32 mode on the PE, ~13 mantissa
bits); softmax runs without max-subtraction (scores are O(5), exp is safe in
fp32) and the normalization is folded in after the PV matmul, whose stationary
operand carries an extra ones-column so the softmax denominator comes out of
the same accumulation for free.
"""

import sys

sys.path.insert(0, "/opt/trn_rl_repo")

import numpy as np

import concourse.bass as bass
import concourse.mybir as mybir
from concourse import bacc, tile
from concourse import bass_utils
from concourse.masks import make_identity

B, T, C = 2, 2048, 1024
NH, HD = 16, 64
R = B * T                 # 4096 global rows
P = 128
NCORES = 8
SCALE = 0.125             # 1/sqrt(HD)
CC = C // P               # 8 contraction chunks
RC = 8                    # row chunks of 512
RCH = R // RC             # 512
QC = 4                    # q chunks of 512 per batch
KT = T // P               # 16 k-tiles of 128 per batch
NEG = -1.0e9

f32 = mybir.dt.float32
f32r = mybir.dt.float32r

_PROGRAM = None


def _build_program(repeat=1, collective=True, num_devices=NCORES):
    nc = bacc.Bacc("TRN2", target_bir_lowering=False, debug=False,
                   num_devices=num_devices)
    xT_ap = nc.dram_tensor("xT", [C, R], f32r, kind="ExternalInput").ap()
    wqkv_ap = nc.dram_tensor("wqkv", [C, 3 * P], f32r, kind="ExternalInput").ap()
    wproj_ap = nc.dram_tensor("wproj", [C, C], f32r, kind="ExternalInput").ap()
    outT_ap = nc.dram_tensor("outT", [C, RCH], f32, kind="ExternalOutput").ap()

    with tile.TileContext(nc) as tc:
        for _rep in range(repeat):
            _emit_body(tc, nc, xT_ap, wqkv_ap, wproj_ap, outT_ap, collective)

    nc.compile()
    return nc


def _emit_body(tc, nc, xT_ap, wqkv_ap, wproj_ap, outT_ap, collective=True):
        with tc.tile_pool(name="const", bufs=1) as const, \
             tc.tile_pool(name="wq", bufs=1) as wqp, \
             tc.tile_pool(name="wp", bufs=1) as wpp, \
             tc.tile_pool(name="xt", bufs=10) as xtp, \
             tc.tile_pool(name="qkv", bufs=1) as qkvp, \
             tc.tile_pool(name="vtmp", bufs=3) as vtmpp, \
             tc.tile_pool(name="vo", bufs=1) as vop, \
             tc.tile_pool(name="yt", bufs=1) as ytp, \
             tc.tile_pool(name="exp", bufs=16) as expp, \
             tc.tile_pool(name="small", bufs=4) as smallp, \
             tc.tile_pool(name="outsb", bufs=2) as outsbp, \
             tc.tile_pool(name="mm512", bufs=2, space="PSUM") as ps_mm, \
             tc.tile_pool(name="ps_tr", bufs=1, space="PSUM") as ps_tr, \
             tc.tile_pool(name="ps_s", bufs=4, space="PSUM") as ps_s, \
             tc.tile_pool(name="ps_y", bufs=1, space="PSUM") as ps_y, \
             tc.tile_pool(name="dram", bufs=1, space="DRAM") as dram:

            # ---- constants -------------------------------------------------
            ident = const.tile([P, P], f32)
            make_identity(nc, ident[:])
            ones = const.tile([P, 1], f32)
            nc.gpsimd.memset(ones[:], 1.0)
            masks = []
            for d in range(4):
                m = const.tile([P, RCH], f32, name=f"mask{d}")
                nc.gpsimd.memset(m[:], 0.0)
                # keep where q - k = f - p - 128*d >= 0, else NEG
                nc.gpsimd.affine_select(
                    out=m[:], in_=m[:], compare_op=mybir.AluOpType.is_ge,
                    fill=NEG, base=-128 * d, pattern=[[1, RCH]],
                    channel_multiplier=-1)
                masks.append(m)

            # ---- weights ---------------------------------------------------
            wq = wqp.tile([P, CC, 3 * P], f32r)
            nc.sync.dma_start(wq[:], wqkv_ap.rearrange("(co ci) n -> ci co n", ci=P))
            wp = wpp.tile([P, CC, C], f32r)
            nc.sync.dma_start(wp[:], wproj_ap.rearrange("(co ci) n -> ci co n", ci=P))

            # ---- phase 1: qkv ---------------------------------------------
            qT = qkvp.tile([P, R], f32r, name="qT")
            kT = qkvp.tile([P, R], f32r, name="kT")
            vo = vop.tile([P, 2 * KT, 130], f32r)  # [V_h0 | 1 | V_h1 | 1] per k-tile
            # ones columns of vo
            nc.vector.tensor_copy(vo[:, :, 64:65], ones[:, None, :].to_broadcast((P, 2 * KT, 1)))
            nc.vector.tensor_copy(vo[:, :, 129:130], ones[:, None, :].to_broadcast((P, 2 * KT, 1)))

            for rc in range(RC):
                xts = []
                for cc in range(CC):
                    xt = xtp.tile([P, RCH], f32r, tag="xt")
                    nc.sync.dma_start(
                        xt[:], xT_ap[P * cc:P * (cc + 1), RCH * rc:RCH * (rc + 1)])
                    xts.append(xt)
                for ct in range(3):
                    ps = ps_mm.tile([P, RCH], f32, tag="mm")
                    for cc in range(CC):
                        nc.tensor.matmul(ps[:], wq[:, cc, P * ct:P * (ct + 1)],
                                         xts[cc][:], start=(cc == 0),
                                         stop=(cc == CC - 1))
                    if ct == 0:
                        nc.vector.tensor_copy(qT[:, RCH * rc:RCH * (rc + 1)], ps[:])
                    elif ct == 1:
                        nc.vector.tensor_copy(kT[:, RCH * rc:RCH * (rc + 1)], ps[:])
                    else:
                        # v^T chunk -> transpose to natural V, pack into vo
                        vt = vtmpp.tile([P, RCH], f32)
                        nc.scalar.copy(vt[:], ps[:])
                        for s in range(RCH // P):
                            kt32 = 4 * rc + s  # global k-tile index (0..31)
                            pst = ps_tr.tile([P, P], f32)
                            nc.tensor.transpose(pst[:], vt[:, P * s:P * (s + 1)],
                                                ident[:])
                            nc.vector.tensor_copy(vo[:, kt32, 0:64], pst[:, 0:64])
                            nc.vector.tensor_copy(vo[:, kt32, 65:129], pst[:, 64:128])

            # ---- phase 2: attention ---------------------------------------
            yT = ytp.tile([P, R], f32r)
            for g in range(B):
                for h in range(2):
                    pr = 64 * h
                    for qc in range(QC):
                        qoff = T * g + RCH * qc
                        nkt = 4 * qc + 4
                        exps = []
                        for kt in range(nkt):
                            koff = T * g + P * kt
                            ps = ps_s.tile([P, RCH], f32)
                            nc.tensor.matmul(
                                ps[:], kT[pr:pr + 64, koff:koff + P],
                                qT[pr:pr + 64, qoff:qoff + RCH],
                                start=True, stop=True)
                            d = kt - 4 * qc
                            if d >= 0:
                                nc.vector.tensor_add(ps[:], ps[:], masks[d][:])
                            e = expp.tile([P, RCH], f32r, tag="exp")
                            nc.scalar.activation(
                                e[:], ps[:], mybir.ActivationFunctionType.Exp,
                                scale=SCALE)
                            exps.append(e)
                        psy = ps_y.tile([65, RCH], f32)
                        for kt in range(nkt):
                            nc.tensor.matmul(
                                psy[:], vo[:, KT * g + kt, 65 * h:65 * h + 65],
                                exps[kt][:], start=(kt == 0),
                                stop=(kt == nkt - 1))
                        rcp = smallp.tile([1, RCH], f32, tag="recip")
                        nc.vector.reciprocal(rcp[:], psy[64:65, :])
                        bc = smallp.tile([64, RCH], f32, tag="bcast")
                        nc.gpsimd.partition_broadcast(bc[:], rcp[:])
                        nc.vector.tensor_mul(yT[pr:pr + 64, qoff:qoff + RCH],
                                             psy[0:64, :], bc[:])

            # ---- all-to-all ------------------------------------------------
            a2a_in = dram.tile([C, RCH], f32r)
            a2a_out = dram.tile([C, RCH], f32r)
            for i in range(NCORES):
                nc.sync.dma_start(a2a_in[P * i:P * (i + 1), :],
                                  yT[:, RCH * i:RCH * (i + 1)])
            kind = collective if isinstance(collective, str) else ("a2a" if collective else "none")
            if kind == "a2a":
                nc.gpsimd.collective_compute(
                    "AllToAll", mybir.AluOpType.bypass,
                    replica_groups=[list(range(NCORES))],
                    ins=[a2a_in.opt()], outs=[a2a_out.opt()])
            elif kind == "a2a_tiny":
                tin = dram.tile([C, 8], f32r, name="tin")
                tout = dram.tile([C, 8], f32r, name="tout")
                nc.sync.dma_start(tin[:], a2a_in[:, 0:8])
                nc.gpsimd.collective_compute(
                    "AllToAll", mybir.AluOpType.bypass,
                    replica_groups=[list(range(NCORES))],
                    ins=[tin.opt()], outs=[tout.opt()])
                nc.sync.dma_start(a2a_out[:, 0:8], tout[:])
                nc.sync.dma_start(a2a_out[:], a2a_in[:])
            elif kind == "ag":
                ag_out = dram.tile([NCORES * P, R], f32r, name="agout")
                ag_in = dram.tile([P, R], f32r, name="agin")
                nc.sync.dma_start(ag_in[:], a2a_in.rearrange("(s p) q -> p (s q)", p=P))
                nc.gpsimd.collective_compute(
                    "AllGather", mybir.AluOpType.bypass,
                    replica_groups=[list(range(NCORES))],
                    ins=[ag_in.opt()], outs=[ag_out.opt()])
                # timing-only: read back a FIXED slice (results wrong off-core0)
                nc.sync.dma_start(a2a_out[:], ag_out.rearrange("(s p) q -> p (s q)", p=P)[:, 0:RCH])
            else:
                nc.sync.dma_start(a2a_out[:], a2a_in[:])

            # ---- phase 3: projection --------------------------------------
            ytms = []
            for cc in range(CC):
                ytm = xtp.tile([P, RCH], f32r, tag="xt")
                nc.sync.dma_start(ytm[:], a2a_out[P * cc:P * (cc + 1), :])
                ytms.append(ytm)
            for ct in range(CC):
                pp = ps_mm.tile([P, RCH], f32, tag="mm")
                for cc in range(CC):
                    nc.tensor.matmul(pp[:], wp[:, cc, P * ct:P * (ct + 1)],
                                     ytms[cc][:], start=(cc == 0),
                                     stop=(cc == CC - 1))
                ot = outsbp.tile([P, RCH], f32)
                if ct % 2 == 0:
                    nc.vector.tensor_copy(ot[:], pp[:])
                else:
                    nc.scalar.copy(ot[:], pp[:])
                nc.sync.dma_start(outT_ap[P * ct:P * (ct + 1), :], ot[:])


def _get_program():
    global _PROGRAM
    if _PROGRAM is None:
        _PROGRAM = _build_program()
    return _PROGRAM


def make_in_maps(x, w_qkv, w_proj):
    """Host-side sharding: build the 8 per-core input maps."""
    x = np.asarray(x, dtype=np.float32)
    w_qkv = np.asarray(w_qkv, dtype=np.float32)
    w_proj = np.asarray(w_proj, dtype=np.float32)
    xT = np.ascontiguousarray(x.reshape(R, C).T)            # (1024, 4096)
    w_proj = np.ascontiguousarray(w_proj)                   # (1024, 1024)
    in_maps = []
    for j in range(NCORES):
        h0 = 2 * j * HD                                     # first head col
        wq = w_qkv[:, h0:h0 + 2 * HD]
        wk = w_qkv[:, C + h0:C + h0 + 2 * HD]
        wv = w_qkv[:, 2 * C + h0:2 * C + h0 + 2 * HD]
        wshard = np.ascontiguousarray(np.concatenate([wq, wk, wv], axis=1))
        in_maps.append({"xT": xT, "wqkv": wshard, "wproj": w_proj})
    return in_maps


def assemble(results):
    """Host-side unshard: concatenate per-core transposed row slices."""
    y = np.empty((R, C), dtype=np.float32)
    for j in range(NCORES):
        y[RCH * j:RCH * (j + 1), :] = results[j]["outT"].T
    return y.reshape(B, T, C)


def kernel(x, w_qkv, w_proj):
    nc = _get_program()
    in_maps = make_in_maps(x, w_qkv, w_proj)
    res = bass_utils.run_bass_kernel_spmd(nc, in_maps,
                                          core_ids=list(range(NCORES)))
    return assemble(res.results)
